# revision 25
# baseline (speedup 1.0000x reference)
"""Trainium2 Bass kernel for nn_MultiHeadAttention (B=8192, D=1024, 16 heads
used only via the softmax scale 1/8).

Strategy (8 NeuronCores, row-sharded attention, two small collectives):
  - Rows (batch axis) of the attention output are sharded: core c owns rows
    [c*1024, (c+1)*1024).
  - Algebraic restructuring removes the K projection and all weight
    transposes:
        E[i, j] = Q_i . K_j = (Wk^T Q_i) . x_j + (Q_i . bk)
    The per-row constant Q_i.bk cancels in softmax, so with
        M' = Wq^T Wk            (from natural-layout weights)
        Z^T = M'^T x^T + (Wk^T bq)   (per-core, local rows only)
    the energy is E^T[j, i] = sum_d x^T[d, j] * Z^T[d, i].
  - Every core receives the FULL x. An fp16 copy of x is staged to DRAM
    chunk-by-chunk (DMA in f32 -> DVE cast -> DMA out), pipelined several
    blocks ahead of the attention loop; x^T tiles are then produced with
    fp16 XBAR DMA-transposes on the sync queue -- no PE transposes, no x
    collective.
  - V = x_loc @ Wv^T is sharded; its rows are AllGathered in two 1 MB bf16
    halves so the first half lands early. The attention loop processes all
    half-0 j-blocks first, then half-1, and runs the energy matmul one
    block ahead of the attn@V matmul, so both collectives hide completely
    behind compute.
  - Attention runs in the transposed-energy ("E^T") layout:
        P^T = exp(E^T * 0.125)           (no max subtraction)
        out_unnorm[i, o] = sum_j P^T[j, i] * V[j, o]
        s[i] = sum_j P^T[j, i]           (matmul against a ones vector)
        out = out_unnorm / s + bv        (bv folded in post-normalization)
  - Energy path runs fp16 (x^T, Z^T, M'), value path bf16 (P needs bf16
    range), weights matmul (M') f32r, PSUM always fp32.
"""

import sys

sys.path.insert(0, "/opt/trn_rl_repo")

import numpy as np

import concourse.bass as bass  # noqa: F401
import concourse.tile as tile
from concourse import bacc, mybir
from concourse.bass_utils import run_bass_kernel_spmd

B = 8192
D = 1024
P = 128
NCORES = 8
R = B // NCORES  # 1024 rows per core
JBLK = 512  # j-block (keys/values) streamed per iteration
NJB = B // JBLK  # 16
DO = D // P  # 8 feature chunks of 128
IC = R // P  # 8 row chunks of 128 per core
STAGE_AHEAD = 4  # x16 chunks staged this many j-blocks ahead
F32 = mybir.dt.float32
F32R = mybir.dt.float32r
F16 = mybir.dt.float16
BF16 = mybir.dt.bfloat16
AF = mybir.ActivationFunctionType
ALU = mybir.AluOpType
SCALE = 0.125  # 1/sqrt(head_dim=64)

# process all half-0 j-blocks first, then half-1 (second AG gets slack)
JB_ORDER = list(range(0, NJB, 2)) + list(range(1, NJB, 2))


def build_program():
    nc = bacc.Bacc(
        "TRN2", target_bir_lowering=False, debug=False, num_devices=NCORES
    )
    x = nc.dram_tensor("x", [B, D], F32, kind="ExternalInput").ap()
    x_loc = nc.dram_tensor("x_loc", [R, D], F32, kind="ExternalInput").ap()
    w_q = nc.dram_tensor("Wq", [D, D], F32, kind="ExternalInput").ap()
    w_k = nc.dram_tensor("Wk", [D, D], F32, kind="ExternalInput").ap()
    w_v = nc.dram_tensor("Wv", [D, D], F32, kind="ExternalInput").ap()
    b_q = nc.dram_tensor("bq", [D], F32, kind="ExternalInput").ap()
    b_v = nc.dram_tensor("bv", [D], F32, kind="ExternalInput").ap()
    out_loc = nc.dram_tensor("out_loc", [R, D], F32, kind="ExternalOutput").ap()

    with tile.TileContext(nc) as tc:
        _body(nc, tc, x, x_loc, w_q, w_k, w_v, b_q, b_v, out_loc)
    nc.compile()
    return nc


def _body(nc, tc, x, x_loc, w_q, w_k, w_v, b_q, b_v, out_loc):
    from contextlib import ExitStack

    outer = ExitStack()
    outer.__enter__()
    # ---- persistent pools (whole kernel) ----
    const_pool = outer.enter_context(tc.tile_pool(name="const", bufs=1))
    ones_f32 = const_pool.tile([P, 2], F32)
    nc.vector.memset(ones_f32, 1.0)
    ones = const_pool.tile([P, 2], BF16)
    nc.vector.tensor_copy(out=ones, in_=ones_f32)
    ones512 = const_pool.tile([1, JBLK], F16)
    nc.vector.memset(ones512, 1.0)
    bq_sb = const_pool.tile([P, DO], F32R)
    nc.sync.dma_start(bq_sb, b_q.rearrange("(oo p) -> p oo", p=P).bitcast(F32R))
    ones_row = const_pool.tile([1, P], F32)
    nc.vector.memset(ones_row, 1.0)
    # broadcast bv across all 128 partitions with a K=1 matmul:
    bv_bc = const_pool.tile([P, D], F32)
    nc.sync.dma_start(bv_bc[0:1, :], b_v[None, :])
    with tc.tile_pool(name="bv_psum", bufs=2, space="PSUM") as bvp:
        for oh in range(2):
            pt = bvp.tile([P, 512], F32, tag="bvp")
            nc.tensor.matmul(
                pt,
                ones_row,
                bv_bc[0:1, oh * 512 : (oh + 1) * 512],
                start=True,
                stop=True,
            )
            nc.vector.tensor_copy(out=bv_bc[:, oh * 512 : (oh + 1) * 512], in_=pt)

    zt_pool = outer.enter_context(tc.tile_pool(name="zt", bufs=1))
    zt = zt_pool.tile([P, DO, R], F16)  # Z^T: [d_in, dd, i]  (2 MB)

    sums_pool = outer.enter_context(tc.tile_pool(name="sums", bufs=1))
    sums_acc = sums_pool.tile([P, 2 * IC], F32)  # per-row exp-sums (even cols)
    rsum = sums_pool.tile([P, 2 * IC], F32)

    # x16 staging pools (live across phase 1 and the attention loop)
    srow_pool = outer.enter_context(tc.tile_pool(name="srow", bufs=1))
    s16_pool = outer.enter_context(tc.tile_pool(name="s16", bufs=2))

    # DRAM scratch
    dram = outer.enter_context(tc.tile_pool(name="dram", bufs=1, space="DRAM"))
    x16_d = dram.tile([B, D], F16)  # fp16 copy of x (16 MB)
    xloc16_d = dram.tile([R, D], F16)  # 2 MB
    wv16_d = dram.tile([D, D], F16)  # 2 MB
    v_loc_h = [dram.tile([R // 2, D], BF16, name=f"vloc{h}") for h in range(2)]
    v_g_h = [
        dram.tile([NCORES, R // 2, D], BF16, addr_space="Shared", name=f"vg{h}")
        for h in range(2)
    ]

    def stage_x16_in(jb):
        """DMA a 512-row chunk of x (f32) into SBUF and cast to fp16."""
        srow = srow_pool.tile([P, JBLK // P, D], F32, tag="srow", name="srow")
        nc.scalar.dma_start(
            srow,
            x[jb * JBLK : (jb + 1) * JBLK, :].rearrange(
                "(jj p) d -> p jj d", p=P
            ),
        )
        s16 = s16_pool.tile([P, JBLK // P, D], F16, tag="s16", name="s16")
        nc.vector.tensor_copy(out=s16, in_=srow)
        return s16

    def stage_x16_out(jb, s16):
        nc.scalar.dma_start(
            x16_d[jb * JBLK : (jb + 1) * JBLK, :].rearrange(
                "(jj p) d -> p jj d", p=P
            ),
            s16,
        )

    # =========================================================
    # Phase 1: staging; M'; Z^T; V (+ two half AllGathers)
    # =========================================================
    with ExitStack() as p1:
        mm_psum = p1.enter_context(tc.tile_pool(name="mm_ps", bufs=4, space="PSUM"))
        g_psum = p1.enter_context(tc.tile_pool(name="g_ps", bufs=1, space="PSUM"))
        st_pool = p1.enter_context(tc.tile_pool(name="stage", bufs=2))

        xt_pool = p1.enter_context(tc.tile_pool(name="xt", bufs=1))
        xt = xt_pool.tile([P, DO, R], F16)  # x_loc^T (2 MB)
        wv_pool = p1.enter_context(tc.tile_pool(name="wv", bufs=1))
        wvt = wv_pool.tile([P, DO, D], F16)  # Wv^T (2 MB)

        # Wq/Wk loads first on the scalar hwdge queue (M' needs them early).
        wq_pool = p1.enter_context(tc.tile_pool(name="wq", bufs=1))
        wq_sb = wq_pool.tile([P, DO, D], F32R)  # Wq rows: [o, oo, d']
        wk_pool = p1.enter_context(tc.tile_pool(name="wk", bufs=1))
        wk_sb = wk_pool.tile([P, DO, D], F32R)  # Wk rows: [o, oo, d]
        nc.scalar.dma_start(
            wq_sb, w_q.rearrange("(oo p) d -> p oo d", p=P).bitcast(F32R)
        )
        nc.scalar.dma_start(
            wk_sb, w_k.rearrange("(oo p) d -> p oo d", p=P).bitcast(F32R)
        )

        # -- stage fp16 copies of x_loc (sync) and Wv (scalar) to DRAM with
        #    single batched DMAs + one DVE cast each, then DMA-transpose
        #    (sync queue only -- the XBAR path) into SBUF.
        with ExitStack() as stg:
            big_pool = stg.enter_context(tc.tile_pool(name="big", bufs=1))
            b16_pool = stg.enter_context(tc.tile_pool(name="b16", bufs=1))
            for src, dst16, q in (
                (x_loc, xloc16_d, nc.sync),
                (w_v, wv16_d, nc.scalar),
            ):
                big = big_pool.tile([P, DO, D], F32, tag="big", name="big")
                q.dma_start(big, src.rearrange("(ic p) d -> p ic d", p=P))
                big16 = b16_pool.tile([P, DO, D], F16, tag="b16", name="big16")
                nc.vector.tensor_copy(out=big16, in_=big)
                q.dma_start(
                    dst16.rearrange("(ic p) d -> p ic d", p=P), big16
                )
            for dd in range(DO):
                nc.sync.dma_start(
                    xt[:, dd, :],
                    xloc16_d[:, dd * P : (dd + 1) * P],
                    transpose=True,
                )
                nc.sync.dma_start(
                    wvt[:, dd, :],
                    wv16_d[:, dd * P : (dd + 1) * P],
                    transpose=True,
                )

        # prime the x16 staging pipeline during phase-1 compute
        for idx in range(min(STAGE_AHEAD, NJB)):
            stage_x16_out(JB_ORDER[idx], stage_x16_in(JB_ORDER[idx]))

        # -- M' = Wq^T Wk from natural-layout weights (no transposes) --
        mp_pool = p1.enter_context(tc.tile_pool(name="mp", bufs=1))
        mp = mp_pool.tile([P, DO, D], F16)  # M'[d', dp, d] (2 MB)
        g_row = const_pool.tile([1, D], F16)  # g = Wk^T bq as a row
        for dp in range(DO):
            for dh in range(2):
                pm = mm_psum.tile([P, 512], F32, tag="mm")
                for oo in range(DO):
                    nc.tensor.matmul(
                        pm,
                        wq_sb[:, oo, dp * P : (dp + 1) * P],
                        wk_sb[:, oo, dh * 512 : (dh + 1) * 512],
                        start=(oo == 0),
                        stop=(oo == DO - 1),
                    )
                nc.vector.tensor_copy(
                    out=mp[:, dp, dh * 512 : (dh + 1) * 512], in_=pm
                )
        # -- g = Wk^T bq as a row vector (wide matmuls only) --
        for dh in range(2):
            pg = g_psum.tile([1, JBLK], F32, tag="g")
            for oo in range(DO):
                nc.tensor.matmul(
                    pg,
                    bq_sb[:, oo : oo + 1],
                    wk_sb[:, oo, dh * 512 : (dh + 1) * 512],
                    start=(oo == 0),
                    stop=(oo == DO - 1),
                )
            nc.vector.tensor_copy(
                out=g_row[:, dh * 512 : (dh + 1) * 512], in_=pg
            )

        # -- Z^T = M'^T x^T + g  (before V so the energy matmuls can start
        #    while the V AllGathers are still in flight) --
        for dd in range(DO):
            for ih in range(R // JBLK):
                pz = mm_psum.tile([P, JBLK], F32, tag="mm")
                for dp in range(DO):
                    nc.tensor.matmul(
                        pz,
                        mp[:, dp, dd * P : (dd + 1) * P],
                        xt[:, dp, ih * JBLK : (ih + 1) * JBLK],
                        start=(dp == 0),
                        stop=False,
                    )
                # += g[d] * 1[i]  (K=1 outer product adds the bq contribution)
                nc.tensor.matmul(
                    pz,
                    g_row[:, dd * P : (dd + 1) * P],
                    ones512,
                    start=False,
                    stop=True,
                )
                nc.vector.tensor_copy(
                    out=zt[:, dd, ih * JBLK : (ih + 1) * JBLK], in_=pz
                )

        # -- V local (natural [j, o]; bias bv deferred to epilogue) in two
        #    halves, each followed by its own AllGather --
        for h in range(2):
            for jx in range(IC // 2):
                jj = h * (IC // 2) + jx
                vst = st_pool.tile([P, D], BF16, tag="vst")
                pv_h = [
                    mm_psum.tile([P, 512], F32, tag="mm", name="pv")
                    for _ in range(2)
                ]
                for dd in range(DO):
                    for oh in range(2):
                        nc.tensor.matmul(
                            pv_h[oh],
                            xt[:, dd, jj * P : (jj + 1) * P],
                            wvt[:, dd, oh * 512 : (oh + 1) * 512],
                            start=(dd == 0),
                            stop=(dd == DO - 1),
                        )
                for oh in range(2):
                    nc.vector.tensor_copy(
                        out=vst[:, oh * 512 : (oh + 1) * 512], in_=pv_h[oh]
                    )
                nc.sync.dma_start(
                    v_loc_h[h][jx * P : (jx + 1) * P, :], vst
                )
            nc.gpsimd.collective_compute(
                "AllGather",
                mybir.AluOpType.bypass,
                replica_groups=[list(range(NCORES))],
                ins=[v_loc_h[h].opt()],
                outs=[v_g_h[h].opt()],
            )

    # =========================================================
    # Phase 2: streamed attention in E^T layout, E one block ahead of O
    # =========================================================
    with ExitStack() as p2:
        oa_pool = p2.enter_context(tc.tile_pool(name="oacc", bufs=1))
        outacc = oa_pool.tile([P, IC, D], F32)  # 4 MB

        xtb_pool = p2.enter_context(tc.tile_pool(name="xtb", bufs=3))
        v_pool = p2.enter_context(tc.tile_pool(name="vtb", bufs=3))
        pt_pool = p2.enter_context(tc.tile_pool(name="ptb", bufs=3))
        e_psum = p2.enter_context(tc.tile_pool(name="e_ps", bufs=4, space="PSUM"))
        o_psum = p2.enter_context(tc.tile_pool(name="o_ps", bufs=3, space="PSUM"))
        s_psum = p2.enter_context(tc.tile_pool(name="s_ps", bufs=1, space="PSUM"))

        def e_phase(jb):
            """energy matmuls + exp for one j-block; returns (ptb, vtb)."""
            rank, half = jb // 2, jb % 2
            xtb = xtb_pool.tile([P, DO, JBLK], F16, tag="xtb", name="xtb")
            for dd in range(DO):
                nc.sync.dma_start(
                    xtb[:, dd, :],
                    x16_d[jb * JBLK : (jb + 1) * JBLK, dd * P : (dd + 1) * P],
                    transpose=True,
                )
            vtb = v_pool.tile([P, JBLK // P, D], BF16, tag="vtb", name="vtb")
            nc.sync.dma_start(
                vtb,
                v_g_h[half][rank].rearrange("(jj p) o -> p jj o", p=P),
            )
            ptb = pt_pool.tile([P, JBLK // P, R], BF16, tag="ptb", name="ptb")
            for jj in range(JBLK // P):
                pe_h = [
                    e_psum.tile([P, JBLK], F32, tag="pe", name="pe")
                    for _ in range(R // JBLK)
                ]
                for dd in range(DO):
                    for ih in range(R // JBLK):
                        nc.tensor.matmul(
                            pe_h[ih],
                            xtb[:, dd, jj * P : (jj + 1) * P],
                            zt[:, dd, ih * JBLK : (ih + 1) * JBLK],
                            start=(dd == 0),
                            stop=(dd == DO - 1),
                        )
                for ih in range(R // JBLK):
                    nc.scalar.activation(
                        ptb[:, jj, ih * JBLK : (ih + 1) * JBLK],
                        pe_h[ih],
                        AF.Exp,
                        scale=SCALE,
                    )
            return ptb, vtb

        def o_phase(ptb, vtb, first):
            """attn@V accumulation + exp-sums for one j-block."""
            ps = s_psum.tile([P, 2 * IC], F32, tag="ps", name="ps")
            for ic in range(IC):
                po_h = [
                    o_psum.tile([P, 512], F32, tag="po", name="po")
                    for _ in range(2)
                ]
                for jj in range(JBLK // P):
                    for oh in range(2):
                        nc.tensor.matmul(
                            po_h[oh],
                            ptb[:, jj, ic * P : (ic + 1) * P],
                            vtb[:, jj, oh * 512 : (oh + 1) * 512],
                            start=(jj == 0),
                            stop=(jj == JBLK // P - 1),
                        )
                    nc.tensor.matmul(
                        ps[:, 2 * ic : 2 * ic + 2],
                        ptb[:, jj, ic * P : (ic + 1) * P],
                        ones,
                        start=(ic == 0 and jj == 0),
                        stop=(ic == IC - 1 and jj == JBLK // P - 1),
                    )
                for oh in range(2):
                    dst = outacc[:, ic, oh * 512 : (oh + 1) * 512]
                    if first:
                        nc.vector.tensor_copy(out=dst, in_=po_h[oh])
                    else:
                        nc.vector.tensor_tensor(dst, po_h[oh], dst, ALU.add)
            if first:
                nc.vector.tensor_copy(out=sums_acc, in_=ps)
            else:
                nc.vector.tensor_tensor(sums_acc, ps, sums_acc, ALU.add)

        pending = None  # (ptb, vtb, first)
        for idx, jb in enumerate(JB_ORDER):
            nxt = idx + STAGE_AHEAD
            if nxt < NJB:
                stage_x16_out(JB_ORDER[nxt], stage_x16_in(JB_ORDER[nxt]))
            ptb, vtb = e_phase(jb)
            if pending is not None:
                o_phase(*pending)
            pending = (ptb, vtb, idx == 0)
        o_phase(*pending)

        # ---- epilogue: normalize, add bv, write out ----
        nc.vector.reciprocal(rsum, sums_acc)
        fin_pool = p2.enter_context(tc.tile_pool(name="fin", bufs=2))
        for ic in range(IC):
            ofin = fin_pool.tile([P, D], F32, tag="ofin")
            nc.vector.tensor_scalar_mul(
                ofin, outacc[:, ic, :], rsum[:, 2 * ic : 2 * ic + 1]
            )
            nc.vector.tensor_tensor(ofin, ofin, bv_bc, ALU.add)
            nc.sync.dma_start(out_loc[ic * P : (ic + 1) * P, :], ofin)

    outer.close()


_NC_CACHE = None


def _get_program():
    global _NC_CACHE
    if _NC_CACHE is None:
        _NC_CACHE = build_program()
    return _NC_CACHE


def _run(inputs, trace=False):
    nc = _get_program()
    x = np.ascontiguousarray(np.asarray(inputs["x"], dtype=np.float32))
    common = {
        "x": x,
        **{
            k: np.ascontiguousarray(np.asarray(inputs[k], dtype=np.float32))
            for k in ("Wq", "Wk", "Wv", "bq", "bv")
        },
    }
    in_maps = [
        {"x_loc": np.ascontiguousarray(x[c * R : (c + 1) * R]), **common}
        for c in range(NCORES)
    ]
    res = run_bass_kernel_spmd(
        nc, in_maps, core_ids=list(range(NCORES)), trace=trace
    )
    out = np.concatenate([res.results[c]["out_loc"] for c in range(NCORES)], axis=0)
    return out.reshape(B, D, 1).astype(np.float32), res


def kernel(**inputs):
    out, _ = _run(inputs, trace=False)
    return out


# revision 38
# speedup vs baseline: 1.1284x; 1.1284x over previous
"""Trainium2 Bass kernel for nn_MultiHeadAttention (B=8192, D=1024, 16 heads
used only via the softmax scale 1/8).

Strategy (8 NeuronCores, row-sharded attention + AllGather collectives):
  - Rows (batch axis) of the attention output are sharded: core c owns rows
    [c*1024, (c+1)*1024).
  - Algebraic restructuring removes the K projection and all Q/K weight
    transposes:
        E[i, j] = Q_i . K_j = (Wk^T Q_i) . x_j + (Q_i . bk)
    The per-row constant Q_i.bk cancels in softmax, so with
        M' = Wq^T Wk          (from natural-layout weights, no transposes)
        Z^T = M'^T x^T + (Wk^T bq)  (per-core, local rows only)
    the energy is E^T[j, i] = sum_d x^T[d, j] * Z^T[d, i].
  - Each core transposes only its local 1024 rows of x; the full x^T and the
    full V (bf16) are assembled with two AllGather collectives that run on
    the TOPSP/SDMA hardware, fully overlapped with the projection matmuls.
  - Attention runs in the transposed-energy ("E^T") layout so no probability
    transpose is needed:
        P^T = exp(E^T * 0.125)           (no max subtraction; |logit| small)
        out_unnorm[i, o] = sum_j P^T[j, i] * V[j, o]
        s[i] = sum_j P^T[j, i]           (matmul against a ones vector)
        out = out_unnorm / s + bv        (bv folded in post-normalization)
  - Big matmuls run in float32r (full-rate streaming at N=512) with fp32
    PSUM accumulation; P/V use bf16.
"""

import sys

sys.path.insert(0, "/opt/trn_rl_repo")

import numpy as np

import concourse.bass as bass  # noqa: F401
import concourse.tile as tile
from concourse import bacc, mybir
from concourse.bass_utils import run_bass_kernel_spmd
from concourse.masks import make_identity

B = 8192
D = 1024
P = 128
NCORES = 8
R = B // NCORES  # 1024 rows per core
JBLK = 512  # j-block (keys/values) streamed per iteration
NJB = B // JBLK  # 16
DO = D // P  # 8 feature chunks of 128
IC = R // P  # 8 row chunks of 128 per core
F32 = mybir.dt.float32
F32R = mybir.dt.float32r
BF16 = mybir.dt.bfloat16
AF = mybir.ActivationFunctionType
ALU = mybir.AluOpType
SCALE = 0.125  # 1/sqrt(head_dim=64)


def build_program():
    nc = bacc.Bacc(
        "TRN2", target_bir_lowering=False, debug=False, num_devices=NCORES
    )
    x_loc = nc.dram_tensor("x_loc", [R, D], F32, kind="ExternalInput").ap()
    w_q = nc.dram_tensor("Wq", [D, D], F32, kind="ExternalInput").ap()
    w_k = nc.dram_tensor("Wk", [D, D], F32, kind="ExternalInput").ap()
    w_v = nc.dram_tensor("Wv", [D, D], F32, kind="ExternalInput").ap()
    b_q = nc.dram_tensor("bq", [D], F32, kind="ExternalInput").ap()
    b_v = nc.dram_tensor("bv", [D], F32, kind="ExternalInput").ap()
    out_loc = nc.dram_tensor("out_loc", [R, D], F32, kind="ExternalOutput").ap()

    with tile.TileContext(nc) as tc:
        _body(nc, tc, x_loc, w_q, w_k, w_v, b_q, b_v, out_loc)
    nc.compile()
    return nc


def _body(nc, tc, x_loc, w_q, w_k, w_v, b_q, b_v, out_loc):
    from contextlib import ExitStack

    outer = ExitStack()
    outer.__enter__()
    # ---- persistent pools (whole kernel) ----
    const_pool = outer.enter_context(tc.tile_pool(name="const", bufs=1))
    identity = const_pool.tile([P, P], F32)
    make_identity(nc, identity)
    ones_f32 = const_pool.tile([P, 2], F32)
    nc.vector.memset(ones_f32, 1.0)
    ones = const_pool.tile([P, 2], BF16)
    nc.vector.tensor_copy(out=ones, in_=ones_f32)
    bq_sb = const_pool.tile([P, DO], F32R)
    nc.sync.dma_start(bq_sb, b_q.rearrange("(oo p) -> p oo", p=P).bitcast(F32R))
    ones_row = const_pool.tile([1, P], F32)
    nc.vector.memset(ones_row, 1.0)
    ones512 = const_pool.tile([1, JBLK], F32)
    nc.vector.memset(ones512, 1.0)
    # broadcast bv across all 128 partitions with a K=1 matmul:
    bv_bc = const_pool.tile([P, D], F32)
    nc.sync.dma_start(bv_bc[0:1, :], b_v[None, :])
    with tc.tile_pool(name="bv_psum", bufs=2, space="PSUM") as bvp:
        for oh in range(2):
            pt = bvp.tile([P, 512], F32, tag="bvp")
            nc.tensor.matmul(
                pt,
                ones_row,
                bv_bc[0:1, oh * 512 : (oh + 1) * 512],
                start=True,
                stop=True,
            )
            nc.vector.tensor_copy(out=bv_bc[:, oh * 512 : (oh + 1) * 512], in_=pt)

    zt_pool = outer.enter_context(tc.tile_pool(name="zt", bufs=1))
    zt = zt_pool.tile([P, DO, R], F32R)  # Z^T: [d_in, dd, i]  (4 MB)

    sums_pool = outer.enter_context(tc.tile_pool(name="sums", bufs=1))
    sums_acc = sums_pool.tile([P, 2 * IC], F32)  # per-row exp-sums (even cols)
    rsum = sums_pool.tile([P, 2 * IC], F32)

    # DRAM scratch: local x^T / V shards + AllGather outputs (Shared).
    dram = outer.enter_context(tc.tile_pool(name="dram", bufs=1, space="DRAM"))
    xt_loc_d = dram.tile([DO, P, R], F32R)  # local x^T shard (4 MB)
    xt_g = dram.tile([NCORES, DO, P, R], F32R, addr_space="Shared")  # 32 MB
    v_loc_d = dram.tile([R, D], BF16)  # local V shard (2 MB)
    v_g = dram.tile([NCORES, R, D], BF16, addr_space="Shared")  # 16 MB

    # =========================================================
    # Phase 1: x^T (local), AG(x^T); M' = Wq^T Wk, Z^T; V, AG(V)
    # =========================================================
    with ExitStack() as p1:
        row_pool = p1.enter_context(tc.tile_pool(name="rows", bufs=2))
        tp_psum = p1.enter_context(tc.tile_pool(name="tp_ps", bufs=2, space="PSUM"))
        mm_psum = p1.enter_context(tc.tile_pool(name="mm_ps", bufs=4, space="PSUM"))
        g_psum = p1.enter_context(tc.tile_pool(name="g_ps", bufs=1, space="PSUM"))
        st_pool = p1.enter_context(tc.tile_pool(name="stage", bufs=2))

        xt_pool = p1.enter_context(tc.tile_pool(name="xt", bufs=1))
        xt = xt_pool.tile([P, DO, R], F32R)  # x_loc^T (4 MB)
        xt_bf = xt_pool.tile([P, DO, R], BF16)  # bf16 copy for V stationary

        # -- transpose local x rows into x^T; ship shard to DRAM + AllGather --
        for ic in range(IC):
            xrow = row_pool.tile([P, D], F32, tag="row")
            nc.sync.dma_start(xrow, x_loc[ic * P : (ic + 1) * P, :])
            for dd in range(DO):
                tp = tp_psum.tile([P, P], F32, tag="tp")
                nc.tensor.transpose(tp, xrow[:, dd * P : (dd + 1) * P], identity)
                nc.vector.tensor_copy(
                    out=xt[:, dd, ic * P : (ic + 1) * P], in_=tp
                )
        nc.vector.tensor_copy(out=xt_bf, in_=xt)
        for dd in range(DO):
            nc.sync.dma_start(xt_loc_d[dd], xt[:, dd, :])
        nc.gpsimd.collective_compute(
            "AllGather",
            mybir.AluOpType.bypass,
            replica_groups=[list(range(NCORES))],
            ins=[xt_loc_d.opt()],
            outs=[xt_g.opt()],
        )

        # -- M' = Wq^T Wk from natural-layout weights (no transposes) --
        mp_pool = p1.enter_context(tc.tile_pool(name="mp", bufs=1))
        mp = mp_pool.tile([P, DO, D], F32R)  # M'[d', dp, d] (4 MB)
        g_row = const_pool.tile([1, D], F32R)  # g = Wk^T bq as a row
        with ExitStack() as wqk:
            wq_pool = wqk.enter_context(tc.tile_pool(name="wq", bufs=1))
            wq_sb = wq_pool.tile([P, DO, D], F32R)  # Wq rows: [o, oo, d']
            wk_pool = wqk.enter_context(tc.tile_pool(name="wk", bufs=1))
            wk_sb = wk_pool.tile([P, DO, D], F32R)  # Wk rows: [o, oo, d]
            # scalar hwdge queue: keeps these off the sync queue, which is
            # busy feeding the x^T shard writes for the AllGather
            nc.scalar.dma_start(
                wq_sb, w_q.rearrange("(oo p) d -> p oo d", p=P).bitcast(F32R)
            )
            nc.scalar.dma_start(
                wk_sb, w_k.rearrange("(oo p) d -> p oo d", p=P).bitcast(F32R)
            )
            for dp in range(DO):
                for dh in range(2):
                    pm = mm_psum.tile([P, 512], F32, tag="mm")
                    for oo in range(DO):
                        nc.tensor.matmul(
                            pm,
                            wq_sb[:, oo, dp * P : (dp + 1) * P],
                            wk_sb[:, oo, dh * 512 : (dh + 1) * 512],
                            start=(oo == 0),
                            stop=(oo == DO - 1),
                        )
                    nc.vector.tensor_copy(
                        out=mp[:, dp, dh * 512 : (dh + 1) * 512], in_=pm
                    )
            # -- g = Wk^T bq as a row vector (wide matmuls only) --
            for dh in range(2):
                pg = g_psum.tile([1, JBLK], F32, tag="g")
                for oo in range(DO):
                    nc.tensor.matmul(
                        pg,
                        bq_sb[:, oo : oo + 1],
                        wk_sb[:, oo, dh * 512 : (dh + 1) * 512],
                        start=(oo == 0),
                        stop=(oo == DO - 1),
                    )
                nc.vector.tensor_copy(
                    out=g_row[:, dh * 512 : (dh + 1) * 512], in_=pg
                )

        # -- Z^T = M'^T x^T + g --
        for dd in range(DO):
            for ih in range(R // JBLK):
                pz = mm_psum.tile([P, JBLK], F32, tag="mm")
                for dp in range(DO):
                    nc.tensor.matmul(
                        pz,
                        mp[:, dp, dd * P : (dd + 1) * P],
                        xt[:, dp, ih * JBLK : (ih + 1) * JBLK],
                        start=(dp == 0),
                        stop=False,
                    )
                # += g[d] * 1[i]  (K=1 outer product adds the bq contribution)
                nc.tensor.matmul(
                    pz,
                    g_row[:, dd * P : (dd + 1) * P],
                    ones512.bitcast(F32R),
                    start=False,
                    stop=True,
                )
                nc.scalar.activation(
                    zt[:, dd, ih * JBLK : (ih + 1) * JBLK],
                    pz,
                    AF.Identity,
                )

        # -- V local (natural [j, o]; bias bv deferred to epilogue), AG(V) --
        wv_pool = p1.enter_context(tc.tile_pool(name="wv", bufs=1))
        wvt = wv_pool.tile([P, DO, D], BF16)  # Wv^T: [d, dd, o] (2 MB)
        for oo in range(DO):
            wrow = row_pool.tile([P, D], F32, tag="row")
            nc.scalar.dma_start(wrow, w_v[oo * P : (oo + 1) * P, :])
            for dd in range(DO):
                tp = tp_psum.tile([P, P], F32, tag="tp")
                nc.tensor.transpose(tp, wrow[:, dd * P : (dd + 1) * P], identity)
                nc.vector.tensor_copy(
                    out=wvt[:, dd, oo * P : (oo + 1) * P], in_=tp
                )
        for jj in range(IC):
            vst = st_pool.tile([P, D], BF16, tag="vst")
            pv_h = [
                mm_psum.tile([P, 512], F32, tag="mm", name="pv") for _ in range(2)
            ]
            for dd in range(DO):
                for oh in range(2):
                    nc.tensor.matmul(
                        pv_h[oh],
                        xt_bf[:, dd, jj * P : (jj + 1) * P],
                        wvt[:, dd, oh * 512 : (oh + 1) * 512],
                        start=(dd == 0),
                        stop=(dd == DO - 1),
                    )
            for oh in range(2):
                nc.vector.tensor_copy(
                    out=vst[:, oh * 512 : (oh + 1) * 512], in_=pv_h[oh]
                )
            nc.scalar.dma_start(v_loc_d[jj * P : (jj + 1) * P, :], vst)
        nc.gpsimd.collective_compute(
            "AllGather",
            mybir.AluOpType.bypass,
            replica_groups=[list(range(NCORES))],
            ins=[v_loc_d.opt()],
            outs=[v_g.opt()],
        )

    # =========================================================
    # Phase 2: streamed attention in E^T layout
    # =========================================================
    with ExitStack() as p2:
        oa_pool = p2.enter_context(tc.tile_pool(name="oacc", bufs=1))
        outacc = oa_pool.tile([P, IC, D], F32)  # 4 MB

        xtb_pool = p2.enter_context(tc.tile_pool(name="xtb", bufs=3))
        v_pool = p2.enter_context(tc.tile_pool(name="vtb", bufs=3))
        pt_pool = p2.enter_context(tc.tile_pool(name="ptb", bufs=3))
        e_psum = p2.enter_context(tc.tile_pool(name="e_ps", bufs=4, space="PSUM"))
        o_psum = p2.enter_context(tc.tile_pool(name="o_ps", bufs=3, space="PSUM"))
        s_psum = p2.enter_context(tc.tile_pool(name="s_ps", bufs=1, space="PSUM"))

        def e_phase(jb):
            """energy matmuls + exp for one j-block; returns ptb."""
            rank, half = jb // 2, jb % 2
            xtb = xtb_pool.tile([P, DO, JBLK], F32R, tag="xtb", name="xtb")
            for dd in range(DO):
                nc.sync.dma_start(
                    xtb[:, dd, :],
                    xt_g[rank, dd, :, half * JBLK : (half + 1) * JBLK],
                )
            ptb = pt_pool.tile([P, JBLK // P, R], BF16, tag="ptb", name="ptb")
            for jj in range(JBLK // P):
                pe_h = [
                    e_psum.tile([P, JBLK], F32, tag="pe", name="pe")
                    for _ in range(R // JBLK)
                ]
                for dd in range(DO):
                    for ih in range(R // JBLK):
                        nc.tensor.matmul(
                            pe_h[ih],
                            xtb[:, dd, jj * P : (jj + 1) * P],
                            zt[:, dd, ih * JBLK : (ih + 1) * JBLK],
                            start=(dd == 0),
                            stop=(dd == DO - 1),
                        )
                for ih in range(R // JBLK):
                    nc.scalar.activation(
                        ptb[:, jj, ih * JBLK : (ih + 1) * JBLK],
                        pe_h[ih],
                        AF.Exp,
                        scale=SCALE,
                    )
            return ptb

        def o_phase(jb, ptb, first):
            """attn@V accumulation + exp-sums for one j-block. The vtb DMA
            goes on the scalar hwdge queue so a wait on the AllGather never
            blocks the xtb reads flowing on the sync queue."""
            rank, half = jb // 2, jb % 2
            vtb = v_pool.tile([P, JBLK // P, D], BF16, tag="vtb", name="vtb")
            nc.scalar.dma_start(
                vtb,
                v_g[rank, half * JBLK : (half + 1) * JBLK, :].rearrange(
                    "(jj p) o -> p jj o", p=P
                ),
            )
            ps = s_psum.tile([P, 2 * IC], F32, tag="ps", name="ps")
            for ic in range(IC):
                po_h = [
                    o_psum.tile([P, 512], F32, tag="po", name="po")
                    for _ in range(2)
                ]
                for jj in range(JBLK // P):
                    for oh in range(2):
                        nc.tensor.matmul(
                            po_h[oh],
                            ptb[:, jj, ic * P : (ic + 1) * P],
                            vtb[:, jj, oh * 512 : (oh + 1) * 512],
                            start=(jj == 0),
                            stop=(jj == JBLK // P - 1),
                        )
                    nc.tensor.matmul(
                        ps[:, 2 * ic : 2 * ic + 2],
                        ptb[:, jj, ic * P : (ic + 1) * P],
                        ones,
                        start=(ic == 0 and jj == 0),
                        stop=(ic == IC - 1 and jj == JBLK // P - 1),
                    )
                for oh in range(2):
                    dst = outacc[:, ic, oh * 512 : (oh + 1) * 512]
                    if first:
                        nc.vector.tensor_copy(out=dst, in_=po_h[oh])
                    else:
                        nc.vector.tensor_tensor(dst, po_h[oh], dst, ALU.add)
            if first:
                nc.vector.tensor_copy(out=sums_acc, in_=ps)
            else:
                nc.vector.tensor_tensor(sums_acc, ps, sums_acc, ALU.add)

        # energy runs one j-block ahead of attn@V so the V AllGather tail
        # hides behind the first two energy phases
        pending = None
        for jb in range(NJB):
            ptb = e_phase(jb)
            if pending is not None:
                o_phase(pending[0], pending[1], pending[0] == 0)
            pending = (jb, ptb)
        o_phase(pending[0], pending[1], pending[0] == 0)

        # ---- epilogue: normalize, add bv, write out ----
        nc.vector.reciprocal(rsum, sums_acc)
        fin_pool = p2.enter_context(tc.tile_pool(name="fin", bufs=2))
        for ic in range(IC):
            ofin = fin_pool.tile([P, D], F32, tag="ofin")
            nc.vector.tensor_scalar_mul(
                ofin, outacc[:, ic, :], rsum[:, 2 * ic : 2 * ic + 1]
            )
            nc.vector.tensor_tensor(ofin, ofin, bv_bc, ALU.add)
            nc.sync.dma_start(out_loc[ic * P : (ic + 1) * P, :], ofin)

    outer.close()


_NC_CACHE = None


def _get_program():
    global _NC_CACHE
    if _NC_CACHE is None:
        _NC_CACHE = build_program()
    return _NC_CACHE


def _run(inputs, trace=False):
    nc = _get_program()
    x = np.ascontiguousarray(np.asarray(inputs["x"], dtype=np.float32))
    common = {
        k: np.ascontiguousarray(np.asarray(inputs[k], dtype=np.float32))
        for k in ("Wq", "Wk", "Wv", "bq", "bv")
    }
    in_maps = [
        {"x_loc": np.ascontiguousarray(x[c * R : (c + 1) * R]), **common}
        for c in range(NCORES)
    ]
    res = run_bass_kernel_spmd(
        nc, in_maps, core_ids=list(range(NCORES)), trace=trace
    )
    out = np.concatenate([res.results[c]["out_loc"] for c in range(NCORES)], axis=0)
    return out.reshape(B, D, 1).astype(np.float32), res


def kernel(**inputs):
    out, _ = _run(inputs, trace=False)
    return out


# revision 40
# speedup vs baseline: 1.1452x; 1.0148x over previous
"""Trainium2 Bass kernel for nn_MultiHeadAttention (B=8192, D=1024, 16 heads
used only via the softmax scale 1/8).

Strategy (8 NeuronCores, row-sharded attention + AllGather collectives):
  - Rows (batch axis) of the attention output are sharded: core c owns rows
    [c*1024, (c+1)*1024).
  - Algebraic restructuring removes the K projection and all Q/K weight
    transposes:
        E[i, j] = Q_i . K_j = (Wk^T Q_i) . x_j + (Q_i . bk)
    The per-row constant Q_i.bk cancels in softmax, so with
        M' = Wq^T Wk          (from natural-layout weights, no transposes)
        Z^T = M'^T x^T + (Wk^T bq)  (per-core, local rows only)
    the energy is E^T[j, i] = sum_d x^T[d, j] * Z^T[d, i].
  - Each core transposes only its local 1024 rows of x; the full x^T and the
    full V (bf16) are assembled with two AllGather collectives that run on
    the TOPSP/SDMA hardware, fully overlapped with the projection matmuls.
  - Attention runs in the transposed-energy ("E^T") layout so no probability
    transpose is needed:
        P^T = exp(E^T * 0.125)           (no max subtraction; |logit| small)
        out_unnorm[i, o] = sum_j P^T[j, i] * V[j, o]
        s[i] = sum_j P^T[j, i]           (matmul against a ones vector)
        out = out_unnorm / s + bv        (bv folded in post-normalization)
  - Big matmuls run in float32r (full-rate streaming at N=512) with fp32
    PSUM accumulation; P/V use bf16.
"""

import sys

sys.path.insert(0, "/opt/trn_rl_repo")

import numpy as np

import concourse.bass as bass  # noqa: F401
import concourse.tile as tile
from concourse import bacc, mybir
from concourse.bass_utils import run_bass_kernel_spmd
from concourse.masks import make_identity

B = 8192
D = 1024
P = 128
NCORES = 8
R = B // NCORES  # 1024 rows per core
JBLK = 512  # j-block (keys/values) streamed per iteration
NJB = B // JBLK  # 16
DO = D // P  # 8 feature chunks of 128
IC = R // P  # 8 row chunks of 128 per core
F32 = mybir.dt.float32
F32R = mybir.dt.float32r
BF16 = mybir.dt.bfloat16
AF = mybir.ActivationFunctionType
ALU = mybir.AluOpType
SCALE = 0.125  # 1/sqrt(head_dim=64)


def build_program():
    nc = bacc.Bacc(
        "TRN2", target_bir_lowering=False, debug=False, num_devices=NCORES
    )
    x_loc = nc.dram_tensor("x_loc", [R, D], F32, kind="ExternalInput").ap()
    w_q = nc.dram_tensor("Wq", [D, D], F32, kind="ExternalInput").ap()
    w_k = nc.dram_tensor("Wk", [D, D], F32, kind="ExternalInput").ap()
    w_v = nc.dram_tensor("Wv", [D, D], F32, kind="ExternalInput").ap()
    b_q = nc.dram_tensor("bq", [D], F32, kind="ExternalInput").ap()
    b_v = nc.dram_tensor("bv", [D], F32, kind="ExternalInput").ap()
    out_loc = nc.dram_tensor("out_loc", [R, D], F32, kind="ExternalOutput").ap()

    with tile.TileContext(nc) as tc:
        _body(nc, tc, x_loc, w_q, w_k, w_v, b_q, b_v, out_loc)
    nc.compile()
    return nc


def _body(nc, tc, x_loc, w_q, w_k, w_v, b_q, b_v, out_loc):
    from contextlib import ExitStack

    outer = ExitStack()
    outer.__enter__()
    # ---- persistent pools (whole kernel) ----
    const_pool = outer.enter_context(tc.tile_pool(name="const", bufs=1))
    identity = const_pool.tile([P, P], F32)
    make_identity(nc, identity)
    ones_f32 = const_pool.tile([P, 2], F32)
    nc.vector.memset(ones_f32, 1.0)
    ones = const_pool.tile([P, 2], BF16)
    nc.vector.tensor_copy(out=ones, in_=ones_f32)
    bq_sb = const_pool.tile([P, DO], F32R)
    nc.sync.dma_start(bq_sb, b_q.rearrange("(oo p) -> p oo", p=P).bitcast(F32R))
    ones_row = const_pool.tile([1, P], F32)
    nc.vector.memset(ones_row, 1.0)
    ones512 = const_pool.tile([1, JBLK], F32)
    nc.vector.memset(ones512, 1.0)
    # broadcast bv across all 128 partitions with a K=1 matmul:
    bv_bc = const_pool.tile([P, D], F32)
    nc.sync.dma_start(bv_bc[0:1, :], b_v[None, :])
    with tc.tile_pool(name="bv_psum", bufs=2, space="PSUM") as bvp:
        for oh in range(2):
            pt = bvp.tile([P, 512], F32, tag="bvp")
            nc.tensor.matmul(
                pt,
                ones_row,
                bv_bc[0:1, oh * 512 : (oh + 1) * 512],
                start=True,
                stop=True,
            )
            nc.vector.tensor_copy(out=bv_bc[:, oh * 512 : (oh + 1) * 512], in_=pt)

    zt_pool = outer.enter_context(tc.tile_pool(name="zt", bufs=1))
    zt = zt_pool.tile([P, DO, R], F32R)  # Z^T: [d_in, dd, i]  (4 MB)

    sums_pool = outer.enter_context(tc.tile_pool(name="sums", bufs=1))
    sums_acc = sums_pool.tile([P, 2 * IC], F32)  # per-row exp-sums (even cols)
    rsum = sums_pool.tile([P, 2 * IC], F32)

    # DRAM scratch: local x^T / V shards + AllGather outputs (Shared).
    dram = outer.enter_context(tc.tile_pool(name="dram", bufs=1, space="DRAM"))
    xt_loc_d = dram.tile([DO, P, R], F32R)  # local x^T shard (4 MB)
    xt_g = dram.tile([NCORES, DO, P, R], F32R, addr_space="Shared")  # 32 MB
    v_loc_d = dram.tile([R, D], BF16)  # local V shard (2 MB)
    v_g = dram.tile([NCORES, R, D], BF16, addr_space="Shared")  # 16 MB

    # =========================================================
    # Phase 1: x^T (local), AG(x^T); M' = Wq^T Wk, Z^T; V, AG(V)
    # =========================================================
    with ExitStack() as p1:
        row_pool = p1.enter_context(tc.tile_pool(name="rows", bufs=2))
        tp_psum = p1.enter_context(tc.tile_pool(name="tp_ps", bufs=2, space="PSUM"))
        mm_psum = p1.enter_context(tc.tile_pool(name="mm_ps", bufs=4, space="PSUM"))
        g_psum = p1.enter_context(tc.tile_pool(name="g_ps", bufs=1, space="PSUM"))
        st_pool = p1.enter_context(tc.tile_pool(name="stage", bufs=2))

        xt_pool = p1.enter_context(tc.tile_pool(name="xt", bufs=1))
        xt = xt_pool.tile([P, DO, R], F32R)  # x_loc^T (4 MB)
        xt_bf = xt_pool.tile([P, DO, R], BF16)  # bf16 copy for V stationary

        # -- transpose local x rows into x^T; ship shard to DRAM + AllGather --
        for ic in range(IC):
            xrow = row_pool.tile([P, D], F32, tag="row")
            nc.sync.dma_start(xrow, x_loc[ic * P : (ic + 1) * P, :])
            for dd in range(DO):
                tp = tp_psum.tile([P, P], F32, tag="tp")
                nc.tensor.transpose(tp, xrow[:, dd * P : (dd + 1) * P], identity)
                nc.vector.tensor_copy(
                    out=xt[:, dd, ic * P : (ic + 1) * P], in_=tp
                )
        nc.vector.tensor_copy(out=xt_bf, in_=xt)
        for dd in range(DO):
            nc.sync.dma_start(xt_loc_d[dd], xt[:, dd, :])
        nc.gpsimd.collective_compute(
            "AllGather",
            mybir.AluOpType.bypass,
            replica_groups=[list(range(NCORES))],
            ins=[xt_loc_d.opt()],
            outs=[xt_g.opt()],
        )

        # -- M' = Wq^T Wk from natural-layout weights (no transposes) --
        mp_pool = p1.enter_context(tc.tile_pool(name="mp", bufs=1))
        mp = mp_pool.tile([P, DO, D], F32R)  # M'[d', dp, d] (4 MB)
        g_row = const_pool.tile([1, D], F32R)  # g = Wk^T bq as a row
        with ExitStack() as wqk:
            wq_pool = wqk.enter_context(tc.tile_pool(name="wq", bufs=1))
            wq_sb = wq_pool.tile([P, DO, D], F32R)  # Wq rows: [o, oo, d']
            wk_pool = wqk.enter_context(tc.tile_pool(name="wk", bufs=1))
            wk_sb = wk_pool.tile([P, DO, D], F32R)  # Wk rows: [o, oo, d]
            # scalar hwdge queue: keeps these off the sync queue, which is
            # busy feeding the x^T shard writes for the AllGather
            nc.scalar.dma_start(
                wq_sb, w_q.rearrange("(oo p) d -> p oo d", p=P).bitcast(F32R)
            )
            nc.scalar.dma_start(
                wk_sb, w_k.rearrange("(oo p) d -> p oo d", p=P).bitcast(F32R)
            )
            for dp in range(DO):
                for dh in range(2):
                    pm = mm_psum.tile([P, 512], F32, tag="mm")
                    for oo in range(DO):
                        nc.tensor.matmul(
                            pm,
                            wq_sb[:, oo, dp * P : (dp + 1) * P],
                            wk_sb[:, oo, dh * 512 : (dh + 1) * 512],
                            start=(oo == 0),
                            stop=(oo == DO - 1),
                        )
                    nc.vector.tensor_copy(
                        out=mp[:, dp, dh * 512 : (dh + 1) * 512], in_=pm
                    )
            # -- g = Wk^T bq as a row vector (wide matmuls only) --
            for dh in range(2):
                pg = g_psum.tile([1, JBLK], F32, tag="g")
                for oo in range(DO):
                    nc.tensor.matmul(
                        pg,
                        bq_sb[:, oo : oo + 1],
                        wk_sb[:, oo, dh * 512 : (dh + 1) * 512],
                        start=(oo == 0),
                        stop=(oo == DO - 1),
                    )
                nc.vector.tensor_copy(
                    out=g_row[:, dh * 512 : (dh + 1) * 512], in_=pg
                )

        # -- Z^T = M'^T x^T + g --
        for dd in range(DO):
            for ih in range(R // JBLK):
                pz = mm_psum.tile([P, JBLK], F32, tag="mm")
                for dp in range(DO):
                    nc.tensor.matmul(
                        pz,
                        mp[:, dp, dd * P : (dd + 1) * P],
                        xt[:, dp, ih * JBLK : (ih + 1) * JBLK],
                        start=(dp == 0),
                        stop=False,
                    )
                # += g[d] * 1[i]  (K=1 outer product adds the bq contribution)
                nc.tensor.matmul(
                    pz,
                    g_row[:, dd * P : (dd + 1) * P],
                    ones512.bitcast(F32R),
                    start=False,
                    stop=True,
                )
                nc.scalar.activation(
                    zt[:, dd, ih * JBLK : (ih + 1) * JBLK],
                    pz,
                    AF.Identity,
                )

        # -- V local (natural [j, o]; bias bv deferred to epilogue), AG(V) --
        wv_pool = p1.enter_context(tc.tile_pool(name="wv", bufs=1))
        wvt = wv_pool.tile([P, DO, D], BF16)  # Wv^T: [d, dd, o] (2 MB)
        for oo in range(DO):
            wrow = row_pool.tile([P, D], F32, tag="row")
            nc.scalar.dma_start(wrow, w_v[oo * P : (oo + 1) * P, :])
            for dd in range(DO):
                tp = tp_psum.tile([P, P], F32, tag="tp")
                nc.tensor.transpose(tp, wrow[:, dd * P : (dd + 1) * P], identity)
                nc.vector.tensor_copy(
                    out=wvt[:, dd, oo * P : (oo + 1) * P], in_=tp
                )
        for jj in range(IC):
            vst = st_pool.tile([P, D], BF16, tag="vst")
            pv_h = [
                mm_psum.tile([P, 512], F32, tag="mm", name="pv") for _ in range(2)
            ]
            for dd in range(DO):
                for oh in range(2):
                    nc.tensor.matmul(
                        pv_h[oh],
                        xt_bf[:, dd, jj * P : (jj + 1) * P],
                        wvt[:, dd, oh * 512 : (oh + 1) * 512],
                        start=(dd == 0),
                        stop=(dd == DO - 1),
                    )
            for oh in range(2):
                nc.vector.tensor_copy(
                    out=vst[:, oh * 512 : (oh + 1) * 512], in_=pv_h[oh]
                )
            nc.scalar.dma_start(v_loc_d[jj * P : (jj + 1) * P, :], vst)
        nc.gpsimd.collective_compute(
            "AllGather",
            mybir.AluOpType.bypass,
            replica_groups=[list(range(NCORES))],
            ins=[v_loc_d.opt()],
            outs=[v_g.opt()],
        )

    # =========================================================
    # Phase 2: streamed attention in E^T layout
    # =========================================================
    with ExitStack() as p2:
        oa_pool = p2.enter_context(tc.tile_pool(name="oacc", bufs=1))
        outacc = oa_pool.tile([P, IC, D], F32)  # 4 MB

        xtb_pool = p2.enter_context(tc.tile_pool(name="xtb", bufs=3))
        v_pool = p2.enter_context(tc.tile_pool(name="vtb", bufs=3))
        pt_pool = p2.enter_context(tc.tile_pool(name="ptb", bufs=4))
        e_psum = p2.enter_context(tc.tile_pool(name="e_ps", bufs=4, space="PSUM"))
        o_psum = p2.enter_context(tc.tile_pool(name="o_ps", bufs=3, space="PSUM"))
        s_psum = p2.enter_context(tc.tile_pool(name="s_ps", bufs=1, space="PSUM"))

        def e_phase(jb):
            """energy matmuls + exp for one j-block; returns ptb."""
            rank, half = jb // 2, jb % 2
            xtb = xtb_pool.tile([P, DO, JBLK], F32R, tag="xtb", name="xtb")
            for dd in range(DO):
                nc.sync.dma_start(
                    xtb[:, dd, :],
                    xt_g[rank, dd, :, half * JBLK : (half + 1) * JBLK],
                )
            ptb = pt_pool.tile([P, JBLK // P, R], BF16, tag="ptb", name="ptb")
            for jj in range(JBLK // P):
                pe_h = [
                    e_psum.tile([P, JBLK], F32, tag="pe", name="pe")
                    for _ in range(R // JBLK)
                ]
                for dd in range(DO):
                    for ih in range(R // JBLK):
                        nc.tensor.matmul(
                            pe_h[ih],
                            xtb[:, dd, jj * P : (jj + 1) * P],
                            zt[:, dd, ih * JBLK : (ih + 1) * JBLK],
                            start=(dd == 0),
                            stop=(dd == DO - 1),
                        )
                for ih in range(R // JBLK):
                    nc.scalar.activation(
                        ptb[:, jj, ih * JBLK : (ih + 1) * JBLK],
                        pe_h[ih],
                        AF.Exp,
                        scale=SCALE,
                    )
            return ptb

        def o_phase(jb, ptb, first):
            """attn@V accumulation + exp-sums for one j-block. The vtb DMA
            goes on the scalar hwdge queue so a wait on the AllGather never
            blocks the xtb reads flowing on the sync queue."""
            rank, half = jb // 2, jb % 2
            vtb = v_pool.tile([P, JBLK // P, D], BF16, tag="vtb", name="vtb")
            nc.scalar.dma_start(
                vtb,
                v_g[rank, half * JBLK : (half + 1) * JBLK, :].rearrange(
                    "(jj p) o -> p jj o", p=P
                ),
            )
            ps = s_psum.tile([P, 2 * IC], F32, tag="ps", name="ps")
            for ic in range(IC):
                po_h = [
                    o_psum.tile([P, 512], F32, tag="po", name="po")
                    for _ in range(2)
                ]
                for jj in range(JBLK // P):
                    for oh in range(2):
                        nc.tensor.matmul(
                            po_h[oh],
                            ptb[:, jj, ic * P : (ic + 1) * P],
                            vtb[:, jj, oh * 512 : (oh + 1) * 512],
                            start=(jj == 0),
                            stop=(jj == JBLK // P - 1),
                        )
                    nc.tensor.matmul(
                        ps[:, 2 * ic : 2 * ic + 2],
                        ptb[:, jj, ic * P : (ic + 1) * P],
                        ones,
                        start=(ic == 0 and jj == 0),
                        stop=(ic == IC - 1 and jj == JBLK // P - 1),
                    )
                for oh in range(2):
                    dst = outacc[:, ic, oh * 512 : (oh + 1) * 512]
                    if first:
                        nc.vector.tensor_copy(out=dst, in_=po_h[oh])
                    else:
                        nc.vector.tensor_tensor(dst, po_h[oh], dst, ALU.add)
            if first:
                nc.vector.tensor_copy(out=sums_acc, in_=ps)
            else:
                nc.vector.tensor_tensor(sums_acc, ps, sums_acc, ALU.add)

        # energy runs two j-blocks ahead of attn@V so the V AllGather tail
        # hides behind the first three energy phases
        from collections import deque

        pending = deque()
        for jb in range(NJB):
            pending.append((jb, e_phase(jb)))
            if len(pending) > 2:
                pjb, pptb = pending.popleft()
                o_phase(pjb, pptb, pjb == 0)
        while pending:
            pjb, pptb = pending.popleft()
            o_phase(pjb, pptb, pjb == 0)

        # ---- epilogue: normalize, add bv, write out ----
        nc.vector.reciprocal(rsum, sums_acc)
        fin_pool = p2.enter_context(tc.tile_pool(name="fin", bufs=2))
        for ic in range(IC):
            ofin = fin_pool.tile([P, D], F32, tag="ofin")
            nc.vector.tensor_scalar_mul(
                ofin, outacc[:, ic, :], rsum[:, 2 * ic : 2 * ic + 1]
            )
            nc.vector.tensor_tensor(ofin, ofin, bv_bc, ALU.add)
            nc.sync.dma_start(out_loc[ic * P : (ic + 1) * P, :], ofin)

    outer.close()


_NC_CACHE = None


def _get_program():
    global _NC_CACHE
    if _NC_CACHE is None:
        _NC_CACHE = build_program()
    return _NC_CACHE


def _run(inputs, trace=False):
    nc = _get_program()
    x = np.ascontiguousarray(np.asarray(inputs["x"], dtype=np.float32))
    common = {
        k: np.ascontiguousarray(np.asarray(inputs[k], dtype=np.float32))
        for k in ("Wq", "Wk", "Wv", "bq", "bv")
    }
    in_maps = [
        {"x_loc": np.ascontiguousarray(x[c * R : (c + 1) * R]), **common}
        for c in range(NCORES)
    ]
    res = run_bass_kernel_spmd(
        nc, in_maps, core_ids=list(range(NCORES)), trace=trace
    )
    out = np.concatenate([res.results[c]["out_loc"] for c in range(NCORES)], axis=0)
    return out.reshape(B, D, 1).astype(np.float32), res


def kernel(**inputs):
    out, _ = _run(inputs, trace=False)
    return out


# revision 46
# speedup vs baseline: 1.1819x; 1.0321x over previous
"""Trainium2 Bass kernel for nn_MultiHeadAttention (B=8192, D=1024, 16 heads
used only via the softmax scale 1/8).

Strategy (8 NeuronCores, row-sharded attention + AllGather collectives):
  - Rows (batch axis) of the attention output are sharded: core c owns rows
    [c*1024, (c+1)*1024).
  - Algebraic restructuring removes the K projection and all Q/K weight
    transposes:
        E[i, j] = Q_i . K_j = (Wk^T Q_i) . x_j + (Q_i . bk)
    The per-row constant Q_i.bk cancels in softmax, so with
        M' = Wq^T Wk          (from natural-layout weights, no transposes)
        Z^T = M'^T x^T + (Wk^T bq)  (per-core, local rows only)
    the energy is E^T[j, i] = sum_d x^T[d, j] * Z^T[d, i].
  - Each core transposes only its local 1024 rows of x; the full x^T and the
    full V (bf16) are assembled with two AllGather collectives that run on
    the TOPSP/SDMA hardware, fully overlapped with the projection matmuls.
  - Attention runs in the transposed-energy ("E^T") layout so no probability
    transpose is needed:
        P^T = exp(E^T * 0.125)           (no max subtraction; |logit| small)
        out_unnorm[i, o] = sum_j P^T[j, i] * V[j, o]
        s[i] = sum_j P^T[j, i]           (matmul against a ones vector)
        out = out_unnorm / s + bv        (bv folded in post-normalization)
  - Big matmuls run in float32r (full-rate streaming at N=512) with fp32
    PSUM accumulation; P/V use bf16.
"""

import sys

sys.path.insert(0, "/opt/trn_rl_repo")

import numpy as np

import concourse.bass as bass  # noqa: F401
import concourse.tile as tile
from concourse import bacc, mybir
from concourse.bass_utils import run_bass_kernel_spmd
from concourse.masks import make_identity

B = 8192
D = 1024
P = 128
NCORES = 8
R = B // NCORES  # 1024 rows per core
JBLK = 512  # j-block (keys/values) streamed per iteration
NJB = B // JBLK  # 16
DO = D // P  # 8 feature chunks of 128
IC = R // P  # 8 row chunks of 128 per core
F32 = mybir.dt.float32
F32R = mybir.dt.float32r
BF16 = mybir.dt.bfloat16
F16 = mybir.dt.float16
AF = mybir.ActivationFunctionType
ALU = mybir.AluOpType
SCALE = 0.125  # 1/sqrt(head_dim=64)


def build_program():
    nc = bacc.Bacc(
        "TRN2", target_bir_lowering=False, debug=False, num_devices=NCORES
    )
    x_loc = nc.dram_tensor("x_loc", [R, D], F32, kind="ExternalInput").ap()
    w_q = nc.dram_tensor("Wq", [D, D], F32, kind="ExternalInput").ap()
    w_k = nc.dram_tensor("Wk", [D, D], F32, kind="ExternalInput").ap()
    w_v = nc.dram_tensor("Wv", [D, D], F32, kind="ExternalInput").ap()
    b_q = nc.dram_tensor("bq", [D], F32, kind="ExternalInput").ap()
    b_v = nc.dram_tensor("bv", [D], F32, kind="ExternalInput").ap()
    out_loc = nc.dram_tensor("out_loc", [R, D], F32, kind="ExternalOutput").ap()

    with tile.TileContext(nc) as tc:
        _body(nc, tc, x_loc, w_q, w_k, w_v, b_q, b_v, out_loc)
    nc.compile()
    return nc


def _body(nc, tc, x_loc, w_q, w_k, w_v, b_q, b_v, out_loc):
    from contextlib import ExitStack

    outer = ExitStack()
    outer.__enter__()
    # ---- persistent pools (whole kernel) ----
    const_pool = outer.enter_context(tc.tile_pool(name="const", bufs=1))
    identity = const_pool.tile([P, P], F32)
    make_identity(nc, identity)
    ones_f32 = const_pool.tile([P, 2], F32)
    nc.vector.memset(ones_f32, 1.0)
    ones = const_pool.tile([P, 2], BF16)
    nc.vector.tensor_copy(out=ones, in_=ones_f32)
    bq_sb = const_pool.tile([P, DO], F32R)
    nc.sync.dma_start(bq_sb, b_q.rearrange("(oo p) -> p oo", p=P).bitcast(F32R))
    ones_row = const_pool.tile([1, P], F32)
    nc.vector.memset(ones_row, 1.0)
    ones512 = const_pool.tile([1, JBLK], F32)
    nc.vector.memset(ones512, 1.0)
    # broadcast bv across all 128 partitions with a K=1 matmul:
    bv_bc = const_pool.tile([P, D], F32)
    nc.sync.dma_start(bv_bc[0:1, :], b_v[None, :])
    with tc.tile_pool(name="bv_psum", bufs=2, space="PSUM") as bvp:
        for oh in range(2):
            pt = bvp.tile([P, 512], F32, tag="bvp")
            nc.tensor.matmul(
                pt,
                ones_row,
                bv_bc[0:1, oh * 512 : (oh + 1) * 512],
                start=True,
                stop=True,
            )
            nc.vector.tensor_copy(out=bv_bc[:, oh * 512 : (oh + 1) * 512], in_=pt)

    zt_pool = outer.enter_context(tc.tile_pool(name="zt", bufs=1))
    zt = zt_pool.tile([P, DO, R], F16)  # Z^T: [d_in, dd, i]  (2 MB)

    sums_pool = outer.enter_context(tc.tile_pool(name="sums", bufs=1))
    sums_acc = sums_pool.tile([P, 2 * IC], F32)  # per-row exp-sums (even cols)
    rsum = sums_pool.tile([P, 2 * IC], F32)

    # DRAM scratch: local x^T / V shards + AllGather outputs (Shared).
    dram = outer.enter_context(tc.tile_pool(name="dram", bufs=1, space="DRAM"))
    xt_loc_d = dram.tile([DO, P, R], F16)  # local x^T shard (2 MB)
    xt_g = dram.tile([NCORES, DO, P, R], F16, addr_space="Shared")  # 16 MB
    v_loc_d = dram.tile([R, D], BF16)  # local V shard (2 MB)
    v_g = dram.tile([NCORES, R, D], BF16, addr_space="Shared")  # 16 MB

    # =========================================================
    # Phase 1: x^T (local), AG(x^T); M' = Wq^T Wk, Z^T; V, AG(V)
    # =========================================================
    with ExitStack() as p1:
        row_pool = p1.enter_context(tc.tile_pool(name="rows", bufs=2))
        tp_psum = p1.enter_context(tc.tile_pool(name="tp_ps", bufs=2, space="PSUM"))
        mm_psum = p1.enter_context(tc.tile_pool(name="mm_ps", bufs=4, space="PSUM"))
        g_psum = p1.enter_context(tc.tile_pool(name="g_ps", bufs=1, space="PSUM"))
        st_pool = p1.enter_context(tc.tile_pool(name="stage", bufs=2))

        xt_pool = p1.enter_context(tc.tile_pool(name="xt", bufs=1))
        xt = xt_pool.tile([P, DO, R], F32R)  # x_loc^T (4 MB)
        xt_bf = xt_pool.tile([P, DO, R], BF16)  # bf16 copy for V stationary

        # -- transpose local x rows into x^T; ship shard to DRAM + AllGather --
        for ic in range(IC):
            xrow = row_pool.tile([P, D], F32, tag="row")
            nc.sync.dma_start(xrow, x_loc[ic * P : (ic + 1) * P, :])
            for dd in range(DO):
                tp = tp_psum.tile([P, P], F32, tag="tp")
                nc.tensor.transpose(tp, xrow[:, dd * P : (dd + 1) * P], identity)
                nc.vector.tensor_copy(
                    out=xt[:, dd, ic * P : (ic + 1) * P], in_=tp
                )
        nc.vector.tensor_copy(out=xt_bf, in_=xt)
        # fp16 copy of x_loc^T: the AllGather ships half the bytes and the
        # energy matmuls run fp16 x fp16
        xt16 = xt_pool.tile([P, DO, R], F16)
        nc.vector.tensor_copy(out=xt16, in_=xt)
        for dd in range(DO):
            nc.sync.dma_start(xt_loc_d[dd], xt16[:, dd, :])
        nc.gpsimd.collective_compute(
            "AllGather",
            mybir.AluOpType.bypass,
            replica_groups=[list(range(NCORES))],
            ins=[xt_loc_d.opt()],
            outs=[xt_g.opt()],
        )

        # -- M' = Wq^T Wk from natural-layout weights (no transposes) --
        mp_pool = p1.enter_context(tc.tile_pool(name="mp", bufs=1))
        mp = mp_pool.tile([P, DO, D], F32R)  # M'[d', dp, d] (4 MB)
        g_row = const_pool.tile([1, D], F32R)  # g = Wk^T bq as a row
        with ExitStack() as wqk:
            wq_pool = wqk.enter_context(tc.tile_pool(name="wq", bufs=1))
            wq_sb = wq_pool.tile([P, DO, D], F32R)  # Wq rows: [o, oo, d']
            wk_pool = wqk.enter_context(tc.tile_pool(name="wk", bufs=1))
            wk_sb = wk_pool.tile([P, DO, D], F32R)  # Wk rows: [o, oo, d]
            # scalar hwdge queue: keeps these off the sync queue, which is
            # busy feeding the x^T shard writes for the AllGather
            nc.scalar.dma_start(
                wq_sb, w_q.rearrange("(oo p) d -> p oo d", p=P).bitcast(F32R)
            )
            nc.scalar.dma_start(
                wk_sb, w_k.rearrange("(oo p) d -> p oo d", p=P).bitcast(F32R)
            )
            for dp in range(DO):
                for dh in range(2):
                    pm = mm_psum.tile([P, 512], F32, tag="mm")
                    for oo in range(DO):
                        nc.tensor.matmul(
                            pm,
                            wq_sb[:, oo, dp * P : (dp + 1) * P],
                            wk_sb[:, oo, dh * 512 : (dh + 1) * 512],
                            start=(oo == 0),
                            stop=(oo == DO - 1),
                        )
                    nc.vector.tensor_copy(
                        out=mp[:, dp, dh * 512 : (dh + 1) * 512], in_=pm
                    )
            # -- g = Wk^T bq as a row vector (wide matmuls only) --
            for dh in range(2):
                pg = g_psum.tile([1, JBLK], F32, tag="g")
                for oo in range(DO):
                    nc.tensor.matmul(
                        pg,
                        bq_sb[:, oo : oo + 1],
                        wk_sb[:, oo, dh * 512 : (dh + 1) * 512],
                        start=(oo == 0),
                        stop=(oo == DO - 1),
                    )
                nc.vector.tensor_copy(
                    out=g_row[:, dh * 512 : (dh + 1) * 512], in_=pg
                )

        # -- Z^T = M'^T x^T + g --
        for dd in range(DO):
            for ih in range(R // JBLK):
                pz = mm_psum.tile([P, JBLK], F32, tag="mm")
                for dp in range(DO):
                    nc.tensor.matmul(
                        pz,
                        mp[:, dp, dd * P : (dd + 1) * P],
                        xt[:, dp, ih * JBLK : (ih + 1) * JBLK],
                        start=(dp == 0),
                        stop=False,
                    )
                # += g[d] * 1[i]  (K=1 outer product adds the bq contribution)
                nc.tensor.matmul(
                    pz,
                    g_row[:, dd * P : (dd + 1) * P],
                    ones512.bitcast(F32R),
                    start=False,
                    stop=True,
                )
                # DVE copy, not scalar.activation: ACT fp16 output corrupts
                nc.vector.tensor_copy(
                    out=zt[:, dd, ih * JBLK : (ih + 1) * JBLK], in_=pz
                )

        # -- V local (natural [j, o]; bias bv deferred to epilogue), AG(V) --
        wv_pool = p1.enter_context(tc.tile_pool(name="wv", bufs=1))
        wvt = wv_pool.tile([P, DO, D], BF16)  # Wv^T: [d, dd, o] (2 MB)
        for oo in range(DO):
            wrow = row_pool.tile([P, D], F32, tag="row")
            nc.scalar.dma_start(wrow, w_v[oo * P : (oo + 1) * P, :])
            for dd in range(DO):
                tp = tp_psum.tile([P, P], F32, tag="tp")
                nc.tensor.transpose(tp, wrow[:, dd * P : (dd + 1) * P], identity)
                nc.vector.tensor_copy(
                    out=wvt[:, dd, oo * P : (oo + 1) * P], in_=tp
                )
        for jj in range(IC):
            vst = st_pool.tile([P, D], BF16, tag="vst")
            pv_h = [
                mm_psum.tile([P, 512], F32, tag="mm", name="pv") for _ in range(2)
            ]
            for dd in range(DO):
                for oh in range(2):
                    nc.tensor.matmul(
                        pv_h[oh],
                        xt_bf[:, dd, jj * P : (jj + 1) * P],
                        wvt[:, dd, oh * 512 : (oh + 1) * 512],
                        start=(dd == 0),
                        stop=(dd == DO - 1),
                    )
            for oh in range(2):
                nc.vector.tensor_copy(
                    out=vst[:, oh * 512 : (oh + 1) * 512], in_=pv_h[oh]
                )
            nc.scalar.dma_start(v_loc_d[jj * P : (jj + 1) * P, :], vst)
        nc.gpsimd.collective_compute(
            "AllGather",
            mybir.AluOpType.bypass,
            replica_groups=[list(range(NCORES))],
            ins=[v_loc_d.opt()],
            outs=[v_g.opt()],
        )

    # =========================================================
    # Phase 2: streamed attention in E^T layout
    # =========================================================
    with ExitStack() as p2:
        oa_pool = p2.enter_context(tc.tile_pool(name="oacc", bufs=1))
        outacc = oa_pool.tile([P, IC, D], F32)  # 4 MB

        xtb_pool = p2.enter_context(tc.tile_pool(name="xtb", bufs=3))
        v_pool = p2.enter_context(tc.tile_pool(name="vtb", bufs=3))
        pt_pool = p2.enter_context(tc.tile_pool(name="ptb", bufs=4))
        e_psum = p2.enter_context(tc.tile_pool(name="e_ps", bufs=4, space="PSUM"))
        o_psum = p2.enter_context(tc.tile_pool(name="o_ps", bufs=3, space="PSUM"))
        s_psum = p2.enter_context(tc.tile_pool(name="s_ps", bufs=1, space="PSUM"))

        def e_phase(jb):
            """energy matmuls + exp for one j-block; returns ptb."""
            rank, half = jb // 2, jb % 2
            xtb = xtb_pool.tile([P, DO, JBLK], F16, tag="xtb", name="xtb")
            for dd in range(DO):
                nc.sync.dma_start(
                    xtb[:, dd, :],
                    xt_g[rank, dd, :, half * JBLK : (half + 1) * JBLK],
                )
            ptb = pt_pool.tile([P, JBLK // P, R], BF16, tag="ptb", name="ptb")
            for jj in range(JBLK // P):
                pe_h = [
                    e_psum.tile([P, JBLK], F32, tag="pe", name="pe")
                    for _ in range(R // JBLK)
                ]
                for dd in range(DO):
                    for ih in range(R // JBLK):
                        nc.tensor.matmul(
                            pe_h[ih],
                            xtb[:, dd, jj * P : (jj + 1) * P],
                            zt[:, dd, ih * JBLK : (ih + 1) * JBLK],
                            start=(dd == 0),
                            stop=(dd == DO - 1),
                        )
                for ih in range(R // JBLK):
                    nc.scalar.activation(
                        ptb[:, jj, ih * JBLK : (ih + 1) * JBLK],
                        pe_h[ih],
                        AF.Exp,
                        scale=SCALE,
                    )
            return ptb

        def o_phase(jb, ptb, first):
            """attn@V accumulation + exp-sums for one j-block. The vtb DMA
            goes on the scalar hwdge queue so a wait on the AllGather never
            blocks the xtb reads flowing on the sync queue."""
            rank, half = jb // 2, jb % 2
            vtb = v_pool.tile([P, JBLK // P, D], BF16, tag="vtb", name="vtb")
            nc.scalar.dma_start(
                vtb,
                v_g[rank, half * JBLK : (half + 1) * JBLK, :].rearrange(
                    "(jj p) o -> p jj o", p=P
                ),
            )
            ps = s_psum.tile([P, 2 * IC], F32, tag="ps", name="ps")
            for ic in range(IC):
                po_h = [
                    o_psum.tile([P, 512], F32, tag="po", name="po")
                    for _ in range(2)
                ]
                for jj in range(JBLK // P):
                    for oh in range(2):
                        nc.tensor.matmul(
                            po_h[oh],
                            ptb[:, jj, ic * P : (ic + 1) * P],
                            vtb[:, jj, oh * 512 : (oh + 1) * 512],
                            start=(jj == 0),
                            stop=(jj == JBLK // P - 1),
                        )
                    nc.tensor.matmul(
                        ps[:, 2 * ic : 2 * ic + 2],
                        ptb[:, jj, ic * P : (ic + 1) * P],
                        ones,
                        start=(ic == 0 and jj == 0),
                        stop=(ic == IC - 1 and jj == JBLK // P - 1),
                    )
                for oh in range(2):
                    dst = outacc[:, ic, oh * 512 : (oh + 1) * 512]
                    if first:
                        nc.vector.tensor_copy(out=dst, in_=po_h[oh])
                    else:
                        nc.vector.tensor_tensor(dst, po_h[oh], dst, ALU.add)
            if first:
                nc.vector.tensor_copy(out=sums_acc, in_=ps)
            else:
                nc.vector.tensor_tensor(sums_acc, ps, sums_acc, ALU.add)

        # energy runs two j-blocks ahead of attn@V so the V AllGather tail
        # hides behind the first three energy phases
        from collections import deque

        pending = deque()
        for jb in range(NJB):
            pending.append((jb, e_phase(jb)))
            if len(pending) > 2:
                pjb, pptb = pending.popleft()
                o_phase(pjb, pptb, pjb == 0)
        while pending:
            pjb, pptb = pending.popleft()
            o_phase(pjb, pptb, pjb == 0)

        # ---- epilogue: normalize, add bv, write out ----
        nc.vector.reciprocal(rsum, sums_acc)
        fin_pool = p2.enter_context(tc.tile_pool(name="fin", bufs=2))
        for ic in range(IC):
            ofin = fin_pool.tile([P, D], F32, tag="ofin")
            nc.vector.tensor_scalar_mul(
                ofin, outacc[:, ic, :], rsum[:, 2 * ic : 2 * ic + 1]
            )
            nc.vector.tensor_tensor(ofin, ofin, bv_bc, ALU.add)
            nc.sync.dma_start(out_loc[ic * P : (ic + 1) * P, :], ofin)

    outer.close()


_NC_CACHE = None


def _get_program():
    global _NC_CACHE
    if _NC_CACHE is None:
        _NC_CACHE = build_program()
    return _NC_CACHE


def _run(inputs, trace=False):
    nc = _get_program()
    x = np.ascontiguousarray(np.asarray(inputs["x"], dtype=np.float32))
    common = {
        k: np.ascontiguousarray(np.asarray(inputs[k], dtype=np.float32))
        for k in ("Wq", "Wk", "Wv", "bq", "bv")
    }
    in_maps = [
        {"x_loc": np.ascontiguousarray(x[c * R : (c + 1) * R]), **common}
        for c in range(NCORES)
    ]
    res = run_bass_kernel_spmd(
        nc, in_maps, core_ids=list(range(NCORES)), trace=trace
    )
    out = np.concatenate([res.results[c]["out_loc"] for c in range(NCORES)], axis=0)
    return out.reshape(B, D, 1).astype(np.float32), res


def kernel(**inputs):
    out, _ = _run(inputs, trace=False)
    return out
